# revision 6
# baseline (speedup 1.0000x reference)
"""Tensor-parallel fused attention kernel for Trainium2 (8 NeuronCores).

Sharding: DP=2 over batch x TP=4 over kv-head pairs. Each core computes
q/k/v projections + RoPE + causal attention + output projection for its
(batch, 2 kv heads) shard in bf16, then a 4-core ReduceScatter combines
the partial output projections; the host assembles the disjoint row
shards into the full [2, 2048, 4096] output.
"""
import sys

for _p in ("/opt/trn_rl_repo", "/root/.axon_site/_ro/trn_rl_repo"):
    if _p not in sys.path:
        sys.path.append(_p)

import math
import numpy as np
import ml_dtypes

import concourse.bass as bass
import concourse.mybir as mybir
import concourse.tile as tile
from concourse import bacc
from concourse import bass_utils
from concourse.masks import make_identity

BF16 = ml_dtypes.bfloat16
FP32 = mybir.dt.float32
BF = mybir.dt.bfloat16

B, S, D = 2, 2048, 4096
R, K, H = 4, 8, 128
N_CORES = 8
TP = 4            # tensor-parallel ways (kv-head axis)
KLOC = K // TP    # kv heads per core = 2
HEADS = R * KLOC  # query heads per core = 8
DT = D // 128     # 32 d-tiles
ST = S // 128     # 16 s-tiles
NG = ST // 4      # 4 supertiles of 512 rows

_CACHE = {}


def _build(causal: bool, debug_no_cc: bool = False):
    nc = bacc.Bacc("TRN2", target_bir_lowering=False, debug=False,
                   enable_asserts=False, num_devices=N_CORES)

    xT = nc.dram_tensor("xT", [D, S], BF, kind="ExternalInput")
    wq = nc.dram_tensor("wq", [D, HEADS * H], BF, kind="ExternalInput")
    wk = nc.dram_tensor("wk", [D, KLOC * H], BF, kind="ExternalInput")
    wv = nc.dram_tensor("wv", [D, KLOC * H], BF, kind="ExternalInput")
    wo = nc.dram_tensor("wo", [HEADS * H, D], BF, kind="ExternalInput")
    cosT = nc.dram_tensor("cosT", [H, S], FP32, kind="ExternalInput")
    sinST = nc.dram_tensor("sinST", [H, S], FP32, kind="ExternalInput")
    if causal:
        maskd = nc.dram_tensor("maskd", [128, S], FP32, kind="ExternalInput")
    else:
        maskf = nc.dram_tensor("maskf", [S, S], FP32, kind="ExternalInput")
    if debug_no_cc:
        out_dbg = nc.dram_tensor("out_dbg", [S, D], FP32, kind="ExternalOutput")
    else:
        out_sh = nc.dram_tensor("out_shard", [S // TP, D], FP32, kind="ExternalOutput")

    with tile.TileContext(nc) as tc:
        with tc.tile_pool(name="persist", bufs=1) as persist, \
             tc.tile_pool(name="dram", bufs=1, space="DRAM") as dram:

            kT_t = [persist.tile([128, S], BF, tag=f"kT{i}", name=f"kT{i}")
                    for i in range(KLOC)]
            v_t = [persist.tile([128, KLOC * H], BF, tag=f"v{i}", name=f"v{i}")
                   for i in range(ST)]
            qT_dram = dram.tile([HEADS * 128, S], BF, tag="qtd", name="qT_dram")
            cc_in = dram.tile([S, D], FP32, tag="ccin", name="cc_in")
            cc_out = dram.tile([S // TP, D], FP32, tag="ccout", name="cc_out")

            # ---------------- Phase 1: projections + rope ----------------
            with tc.tile_pool(name="p1", bufs=1) as p1, \
                 tc.tile_pool(name="p1ps", bufs=1, space="PSUM") as p1ps:
                ct = p1.tile([H, S], FP32, tag="ct")
                nc.sync.dma_start(ct[:], cosT.ap())
                st = p1.tile([H, S], FP32, tag="st")
                nc.sync.dma_start(st[:], sinST.ap())
                wv_sb = p1.tile([128, DT * KLOC * H], BF, tag="wvsb")
                nc.sync.dma_start(
                    wv_sb[:].rearrange("p (a h) -> p a h", a=DT),
                    wv.ap().rearrange("(a p) h -> p a h", p=128))

                for half in range(2):
                    scols = (half * (S // 2), (half + 1) * (S // 2))
                    xth = p1.tile([128, DT * (S // 2)], BF, tag="xth")
                    nc.sync.dma_start(
                        xth[:].rearrange("p (a s) -> p a s", a=DT),
                        xT.ap().rearrange("(a p) s -> p a s", p=128)[:, :, scols[0]:scols[1]])

                    # q (8 head-tiles) then k (KLOC head-tiles)
                    for h in range(HEADS + KLOC):
                        wsrc = wq.ap()[:, (h * 128):(h + 1) * 128] if h < HEADS \
                            else wk.ap()[:, ((h - HEADS) * 128):((h - HEADS) + 1) * 128]
                        wslab = p1.tile([128, DT * 128], BF, tag="wslab", bufs=2)
                        nc.sync.dma_start(
                            wslab[:].rearrange("p (a q) -> p a q", a=DT),
                            wsrc.rearrange("(a p) q -> p a q", p=128))
                        for sc in range(2):  # 512-wide chunks within the half
                            lo = sc * 512
                            qp = p1ps.tile([128, 512], FP32, tag="qp", bufs=3)
                            for d in range(DT):
                                nc.tensor.matmul(
                                    qp[:],
                                    lhsT=wslab[:, d * 128:(d + 1) * 128],
                                    rhs=xth[:, d * (S // 2) + lo: d * (S // 2) + lo + 512],
                                    start=(d == 0), stop=(d == DT - 1))
                            # rope: out = qp*cos + rot(qp)*sin_signed
                            gcol = scols[0] + lo
                            t1 = p1.tile([128, 512], FP32, tag="t1", bufs=2)
                            nc.vector.tensor_mul(t1[:], qp[:], ct[:, gcol:gcol + 512])
                            t2 = p1.tile([128, 512], FP32, tag="t2", bufs=2)
                            nc.vector.tensor_mul(t2[0:64, :], qp[64:128, :],
                                                 st[0:64, gcol:gcol + 512])
                            nc.vector.tensor_mul(t2[64:128, :], qp[0:64, :],
                                                 st[64:128, gcol:gcol + 512])
                            if h < HEADS:
                                robf = p1.tile([128, 512], BF, tag="robf", bufs=2)
                                nc.vector.tensor_add(robf[:], t1[:], t2[:])
                                nc.sync.dma_start(
                                    qT_dram[h * 128:(h + 1) * 128, gcol:gcol + 512],
                                    robf[:])
                            else:
                                nc.vector.tensor_add(
                                    kT_t[h - HEADS][:, gcol:gcol + 512], t1[:], t2[:])

                    # v projection for the 8 s-tiles of this half
                    for stl in range(ST // 2):
                        sti = half * (ST // 2) + stl
                        vp = p1ps.tile([128, KLOC * H], FP32, tag="vp", bufs=2)
                        for d in range(DT):
                            nc.tensor.matmul(
                                vp[:],
                                lhsT=xth[:, d * (S // 2) + stl * 128: d * (S // 2) + (stl + 1) * 128],
                                rhs=wv_sb[:, d * KLOC * H:(d + 1) * KLOC * H],
                                start=(d == 0), stop=(d == DT - 1))
                        nc.scalar.copy(v_t[sti][:], vp[:])

            # ---------------- Phase 2: attention + out-proj ----------------
            with tc.tile_pool(name="p2", bufs=1) as p2, \
                 tc.tile_pool(name="p2ps", bufs=1, space="PSUM") as p2ps:
                wo_sb = [p2.tile([128, D], BF, tag=f"wo{i}", name=f"wo{i}")
                         for i in range(HEADS)]
                for i in range(HEADS):
                    nc.sync.dma_start(wo_sb[i][:], wo.ap()[i * 128:(i + 1) * 128, :])
                ident = p2.tile([128, 128], BF, tag="ident")
                make_identity(nc, ident[:])
                if causal:
                    md = p2.tile([128, S], FP32, tag="maskd")
                    nc.sync.dma_start(md[:], maskd.ap())

                for g in range(NG):
                    qg = [p2.tile([128, 512], BF, tag="qg", bufs=16, name=f"qg{g}_{h}")
                          for h in range(HEADS)]
                    for h in range(HEADS):
                        nc.sync.dma_start(
                            qg[h][:], qT_dram[h * 128:(h + 1) * 128, g * 512:(g + 1) * 512])
                    if not causal:
                        mrow = [p2.tile([128, S], FP32, tag="mrow", bufs=8,
                                        name=f"mrow{g}_{it}") for it in range(4)]
                        for it in range(4):
                            i = 4 * g + it
                            nc.sync.dma_start(mrow[it][:], maskf.ap()[i * 128:(i + 1) * 128, :])

                    yT_sb = [p2.tile([128, 512], BF, tag=f"yt{h}", bufs=2,
                                     name=f"yt{g}_{h}") for h in range(HEADS)]
                    for h in range(HEADS):
                        kv = h % KLOC
                        nquad = g + 1 if causal else NG
                        pTq = [p2.tile([128, 2048], BF, tag=f"ptq{q}", bufs=2,
                                       name=f"ptq{g}_{h}_{q}") for q in range(nquad)]
                        for it in range(4):
                            i = 4 * g + it
                            nsk = (i + 1) * 128 if causal else S
                            prow = p2.tile([128, S], BF, tag="prow", bufs=2)
                            sums = []
                            nch = (nsk + 1023) // 1024
                            for c in range(nch):
                                w = min(1024, nsk - c * 1024)
                                sp = p2ps.tile([128, 1024], FP32, tag="sp", bufs=2)
                                for cc in range((w + 511) // 512):
                                    ww = min(512, w - cc * 512)
                                    o = cc * 512
                                    nc.tensor.matmul(
                                        sp[:, o:o + ww],
                                        lhsT=qg[h][:, it * 128:(it + 1) * 128],
                                        rhs=kT_t[kv][:, c * 1024 + o: c * 1024 + o + ww],
                                        start=True, stop=True)
                                if causal:
                                    dlo = nsk - 128
                                    if c * 1024 <= dlo < c * 1024 + w:
                                        o = dlo - c * 1024
                                        nc.vector.tensor_add(
                                            sp[:, o:o + 128], sp[:, o:o + 128],
                                            md[:, i * 128:(i + 1) * 128])
                                else:
                                    nc.vector.tensor_add(
                                        sp[:, :w], sp[:, :w],
                                        mrow[it][:, c * 1024: c * 1024 + w])
                                sm = p2.tile([128, 1], FP32, tag="sm", bufs=8)
                                nc.scalar.activation(
                                    prow[:, c * 1024: c * 1024 + w], sp[:, :w],
                                    mybir.ActivationFunctionType.Exp, accum_out=sm[:])
                                sums.append(sm)
                            if nch == 2:
                                tot = p2.tile([128, 1], FP32, tag="tot", bufs=4)
                                nc.vector.tensor_add(tot[:], sums[0][:], sums[1][:])
                            else:
                                tot = sums[0]
                            rc = p2.tile([128, 1], FP32, tag="rc", bufs=4)
                            nc.vector.reciprocal(rc[:], tot[:])
                            nc.vector.tensor_scalar_mul(prow[:, :nsk], prow[:, :nsk], rc[:])
                            # transpose p blocks (j <= i if causal) into quad strips
                            jtop = i if causal else ST - 1
                            for q in range(jtop // 4 + 1):
                                jlo, jhi = 4 * q, min(4 * q + 3, jtop)
                                nq = jhi - jlo + 1
                                tpp = p2ps.tile([128, 512], BF, tag="tp", bufs=2)
                                for j in range(jlo, jhi + 1):
                                    nc.tensor.transpose(
                                        tpp[:, (j - jlo) * 128:(j - jlo + 1) * 128],
                                        prow[:, j * 128:(j + 1) * 128], ident[:])
                                pt_dst = pTq[q][:].rearrange("p (a b) -> p a b", a=4)[
                                    :, 0:nq, it * 128:(it + 1) * 128]
                                pt_src = tpp[:, :nq * 128].rearrange(
                                    "p (a b) -> p a b", b=128)
                                if (it + q) % 2:
                                    nc.scalar.copy(pt_dst, pt_src)
                                else:
                                    nc.vector.tensor_copy(pt_dst, pt_src)
                        # y^T accumulation over sk-tiles
                        yp = p2ps.tile([128, 512], FP32, tag="yp", bufs=2)
                        jmax = 4 * g + 4 if causal else ST
                        for j in range(jmax):
                            lo = max(0, j - 4 * g) * 128 if causal else 0
                            nc.tensor.matmul(
                                yp[:, lo:512],
                                lhsT=v_t[j][:, kv * H:(kv + 1) * H],
                                rhs=pTq[j // 4][:, (j % 4) * 512 + lo: (j % 4) * 512 + 512],
                                start=(j == 0), stop=(j == jmax - 1))
                        nc.scalar.copy(yT_sb[h][:], yp[:])

                    # out-projection for this supertile
                    for it in range(4):
                        i = 4 * g + it
                        for dc in range(8):
                            op = p2ps.tile([128, 1024], FP32, tag="sp", bufs=2)
                            for hh in range(HEADS):
                                nc.tensor.matmul(
                                    op[:, 0:512],
                                    lhsT=yT_sb[hh][:, it * 128:(it + 1) * 128],
                                    rhs=wo_sb[hh][:, dc * 512:(dc + 1) * 512],
                                    start=(hh == 0), stop=(hh == HEADS - 1))
                            oev = p2.tile([128, 512], FP32, tag="oev", bufs=4)
                            if dc % 2:
                                nc.scalar.copy(oev[:], op[:, 0:512])
                            else:
                                nc.vector.tensor_copy(oev[:], op[:, 0:512])
                            nc.sync.dma_start(
                                cc_in[i * 128:(i + 1) * 128, dc * 512:(dc + 1) * 512],
                                oev[:])

                if debug_no_cc:
                    nc.sync.dma_start(out_dbg.ap(), cc_in[:])
                if debug_no_cc != "nocc":
                    nc.gpsimd.collective_compute(
                        "ReduceScatter", mybir.AluOpType.add,
                        replica_groups=[[0, 1, 2, 3], [4, 5, 6, 7]],
                        ins=[cc_in.opt()], outs=[cc_out.opt()])
                    nc.sync.dma_start((out_dbg.ap()[0:S // TP, :] if debug_no_cc
                                       else out_sh.ap()), cc_out[:])

    nc.compile()
    return nc


_CANON_MASK = None


def _is_causal(mask: np.ndarray) -> bool:
    global _CANON_MASK
    if _CANON_MASK is None:
        _CANON_MASK = np.triu(np.full((S, S), -1e9, dtype=np.float32), k=1)
    return mask.shape == (S, S) and np.array_equal(mask, _CANON_MASK)


def _prepare(x, wq, wk, wv, wo, mask, sin, cos):
    causal = _is_causal(np.asarray(mask, dtype=np.float32))
    if causal not in _CACHE:
        _CACHE[causal] = _build(causal)
    nc = _CACHE[causal]

    x = np.asarray(x, dtype=np.float32)
    scale = np.float32(H ** -0.5)
    cosT = np.ascontiguousarray(np.asarray(cos, np.float32).T)          # [H, S]
    sinT = np.asarray(sin, np.float32).T.copy()                          # [H, S]
    sinT[0:H // 2] = -sinT[0:H // 2]                                     # signed
    # per-core weight shards; head order = r-major over local kv heads
    in_maps = []
    for c in range(N_CORES):
        b, tp = c // TP, c % TP
        ks = slice(tp * KLOC, (tp + 1) * KLOC)
        m = {
            "xT": np.ascontiguousarray(x[b].T).astype(BF16),
            "wq": np.asarray(wq, np.float32)[:, :, ks, :].reshape(D, HEADS * H).astype(BF16),
            "wk": (np.asarray(wk, np.float32)[:, ks, :] * scale).reshape(D, KLOC * H).astype(BF16),
            "wv": np.asarray(wv, np.float32)[:, ks, :].reshape(D, KLOC * H).astype(BF16),
            "wo": np.asarray(wo, np.float32)[:, ks, :, :].reshape(HEADS * H, D).astype(BF16),
            "cosT": cosT,
            "sinST": sinT,
        }
        if causal:
            md = np.empty((128, S), np.float32)
            for i in range(ST):
                md[:, i * 128:(i + 1) * 128] = mask[i * 128:(i + 1) * 128,
                                                    i * 128:(i + 1) * 128]
            m["maskd"] = md
        else:
            m["maskf"] = np.asarray(mask, np.float32)
        in_maps.append(m)
    return nc, in_maps


def _assemble(results):
    out = np.empty((B, S, D), dtype=np.float32)
    for c in range(N_CORES):
        b, tp = c // TP, c % TP
        out[b, tp * (S // TP):(tp + 1) * (S // TP), :] = results[c]["out_shard"]
    return out


def kernel(x, wq, wk, wv, wo, mask, sin, cos):
    nc, in_maps = _prepare(x, wq, wk, wv, wo, mask, sin, cos)
    res = bass_utils.run_bass_kernel_spmd(nc, in_maps, core_ids=list(range(N_CORES)))
    return _assemble(res.results)


def _traced_run(x, wq, wk, wv, wo, mask, sin, cos):
    """Like kernel() but with NTFF tracing; returns BassKernelResults."""
    nc, in_maps = _prepare(x, wq, wk, wv, wo, mask, sin, cos)
    res = bass_utils.run_bass_kernel_spmd(nc, in_maps, core_ids=list(range(N_CORES)),
                                          trace=True)
    res.full_output = _assemble(res.results)
    return res


# revision 8
# speedup vs baseline: 1.2673x; 1.2673x over previous
"""Tensor-parallel fused attention kernel for Trainium2 (8 NeuronCores).

Sharding: DP=2 over batch x TP=4 over kv-head pairs. Each core computes
q/k/v projections + RoPE + causal attention + output projection for its
(batch, 2 kv heads) shard in bf16, then a 4-core ReduceScatter combines
the partial output projections; the host assembles the disjoint row
shards into the full [2, 2048, 4096] output.
"""
import sys

for _p in ("/opt/trn_rl_repo", "/root/.axon_site/_ro/trn_rl_repo"):
    if _p not in sys.path:
        sys.path.append(_p)

import math
import numpy as np
import ml_dtypes

import concourse.bass as bass
import concourse.mybir as mybir
import concourse.tile as tile
from concourse import bacc
from concourse import bass_utils
from concourse.masks import make_identity

BF16 = ml_dtypes.bfloat16
FP32 = mybir.dt.float32
BF = mybir.dt.bfloat16

B, S, D = 2, 2048, 4096
R, K, H = 4, 8, 128
N_CORES = 8
TP = 4            # tensor-parallel ways (kv-head axis)
KLOC = K // TP    # kv heads per core = 2
HEADS = R * KLOC  # query heads per core = 8
DT = D // 128     # 32 d-tiles
ST = S // 128     # 16 s-tiles
NG = ST // 4      # 4 supertiles of 512 rows

_CACHE = {}


def _build(causal: bool):
    nc = bacc.Bacc("TRN2", target_bir_lowering=False, debug=False,
                   enable_asserts=False, num_devices=N_CORES)

    xP = nc.dram_tensor("xP", [128, 2 * DT * (S // 2)], BF, kind="ExternalInput")
    wq = nc.dram_tensor("wq", [HEADS * 128, DT * 128], BF, kind="ExternalInput")
    wk = nc.dram_tensor("wk", [KLOC * 128, DT * 128], BF, kind="ExternalInput")
    wv = nc.dram_tensor("wv", [128, DT * KLOC * H], BF, kind="ExternalInput")
    wo = nc.dram_tensor("wo", [HEADS * H, D], BF, kind="ExternalInput")
    cosT = nc.dram_tensor("cosT", [H, S], FP32, kind="ExternalInput")
    sinST = nc.dram_tensor("sinST", [H, S], FP32, kind="ExternalInput")
    if causal:
        maskd = nc.dram_tensor("maskd", [128, S], FP32, kind="ExternalInput")
    else:
        maskf = nc.dram_tensor("maskf", [S, S], FP32, kind="ExternalInput")
    out_sh = nc.dram_tensor("out_shard", [S // TP, D], FP32, kind="ExternalOutput")

    with tile.TileContext(nc) as tc:
        with tc.tile_pool(name="persist", bufs=1) as persist, \
             tc.tile_pool(name="dram", bufs=1, space="DRAM") as dram:

            kT_t = [persist.tile([128, S], BF, tag=f"kT{i}", name=f"kT{i}")
                    for i in range(KLOC)]
            v_t = [persist.tile([128, KLOC * H], BF, tag=f"v{i}", name=f"v{i}")
                   for i in range(ST)]
            qT_dram = dram.tile([HEADS * 128, S], BF, tag="qtd", name="qT_dram")
            cc_in = [dram.tile([512, D], FP32, tag=f"ccin{g}", name=f"cc_in{g}")
                     for g in range(NG)]
            cc_out = [dram.tile([128, D], FP32, tag=f"ccout{g}", name=f"cc_out{g}")
                      for g in range(NG)]

            # ---------------- Phase 1: projections + rope ----------------
            with tc.tile_pool(name="p1", bufs=1) as p1, \
                 tc.tile_pool(name="p1ps", bufs=1, space="PSUM") as p1ps:
                ct = p1.tile([H, S], FP32, tag="ct")
                nc.sync.dma_start(ct[:], cosT.ap())
                st = p1.tile([H, S], FP32, tag="st")
                nc.sync.dma_start(st[:], sinST.ap())
                wv_sb = p1.tile([128, DT * KLOC * H], BF, tag="wvsb")
                nc.sync.dma_start(wv_sb[:], wv.ap())

                for half in range(2):
                    scols = (half * (S // 2), (half + 1) * (S // 2))
                    xth_t = [p1.tile([128, 8 * (S // 2)], BF, tag="xth", bufs=8,
                                     name=f"xth{half}_{qq}") for qq in range(4)]
                    for qq in range(4):
                        nc.sync.dma_start(
                            xth_t[qq][:],
                            xP.ap()[:, (half * DT + qq * 8) * (S // 2):
                                       (half * DT + (qq + 1) * 8) * (S // 2)])

                    def xth(d, a, b):
                        return xth_t[d // 8][:, (d % 8) * (S // 2) + a:
                                             (d % 8) * (S // 2) + b]

                    # q (8 head-tiles) then k (KLOC head-tiles)
                    for h in range(HEADS + KLOC):
                        wsrc = wq.ap()[h * 128:(h + 1) * 128, :] if h < HEADS \
                            else wk.ap()[(h - HEADS) * 128:(h - HEADS + 1) * 128, :]
                        wslab = p1.tile([128, DT * 128], BF, tag="wslab", bufs=2)
                        nc.sync.dma_start(wslab[:], wsrc)
                        for sc in range(2):  # 512-wide chunks within the half
                            lo = sc * 512
                            qp = p1ps.tile([128, 512], FP32, tag="qp", bufs=3)
                            for d in range(DT):
                                nc.tensor.matmul(
                                    qp[:],
                                    lhsT=wslab[:, d * 128:(d + 1) * 128],
                                    rhs=xth(d, lo, lo + 512),
                                    start=(d == 0), stop=(d == DT - 1))
                            # rope: out = qp*cos + rot(qp)*sin_signed
                            gcol = scols[0] + lo
                            t1 = p1.tile([128, 512], FP32, tag="t1", bufs=2)
                            nc.vector.tensor_mul(t1[:], qp[:], ct[:, gcol:gcol + 512])
                            t2 = p1.tile([128, 512], FP32, tag="t2", bufs=2)
                            nc.vector.tensor_mul(t2[0:64, :], qp[64:128, :],
                                                 st[0:64, gcol:gcol + 512])
                            nc.vector.tensor_mul(t2[64:128, :], qp[0:64, :],
                                                 st[64:128, gcol:gcol + 512])
                            if h < HEADS:
                                robf = p1.tile([128, 512], BF, tag="robf", bufs=2)
                                nc.vector.tensor_add(robf[:], t1[:], t2[:])
                                nc.sync.dma_start(
                                    qT_dram[h * 128:(h + 1) * 128, gcol:gcol + 512],
                                    robf[:])
                            else:
                                nc.vector.tensor_add(
                                    kT_t[h - HEADS][:, gcol:gcol + 512], t1[:], t2[:])

                    # v projection for the 8 s-tiles of this half
                    for stl in range(ST // 2):
                        sti = half * (ST // 2) + stl
                        vp = p1ps.tile([128, KLOC * H], FP32, tag="vp", bufs=2)
                        for d in range(DT):
                            nc.tensor.matmul(
                                vp[:],
                                lhsT=xth(d, stl * 128, (stl + 1) * 128),
                                rhs=wv_sb[:, d * KLOC * H:(d + 1) * KLOC * H],
                                start=(d == 0), stop=(d == DT - 1))
                        nc.scalar.copy(v_t[sti][:], vp[:])

            # ---------------- Phase 2: attention + out-proj ----------------
            with tc.tile_pool(name="p2", bufs=1) as p2, \
                 tc.tile_pool(name="p2ps", bufs=1, space="PSUM") as p2ps:
                wo_sb = [p2.tile([128, D], BF, tag=f"wo{i}", name=f"wo{i}")
                         for i in range(HEADS)]
                for i in range(HEADS):
                    nc.sync.dma_start(wo_sb[i][:], wo.ap()[i * 128:(i + 1) * 128, :])
                ident = p2.tile([128, 128], BF, tag="ident")
                make_identity(nc, ident[:])
                if causal:
                    md = p2.tile([128, S], FP32, tag="maskd")
                    nc.sync.dma_start(md[:], maskd.ap())

                for g in range(NG):
                    qg = [p2.tile([128, 512], BF, tag="qg", bufs=16, name=f"qg{g}_{h}")
                          for h in range(HEADS)]
                    for h in range(HEADS):
                        nc.sync.dma_start(
                            qg[h][:], qT_dram[h * 128:(h + 1) * 128, g * 512:(g + 1) * 512])
                    if not causal:
                        mrow = [p2.tile([128, S], FP32, tag="mrow", bufs=8,
                                        name=f"mrow{g}_{it}") for it in range(4)]
                        for it in range(4):
                            i = 4 * g + it
                            nc.sync.dma_start(mrow[it][:], maskf.ap()[i * 128:(i + 1) * 128, :])

                    yT_sb = [p2.tile([128, 512], BF, tag=f"yt{h}", bufs=2,
                                     name=f"yt{g}_{h}") for h in range(HEADS)]
                    for h in range(HEADS):
                        kv = h % KLOC
                        nquad = g + 1 if causal else NG
                        pTq = [p2.tile([128, 2048], BF, tag=f"ptq{q}", bufs=2,
                                       name=f"ptq{g}_{h}_{q}") for q in range(nquad)]
                        for it in range(4):
                            i = 4 * g + it
                            nsk = (i + 1) * 128 if causal else S
                            prow = p2.tile([128, S], BF, tag="prow", bufs=2)
                            sums = []
                            nch = (nsk + 1023) // 1024
                            for c in range(nch):
                                w = min(1024, nsk - c * 1024)
                                sp = p2ps.tile([128, 1024], FP32, tag="sp", bufs=2)
                                for cc in range((w + 511) // 512):
                                    ww = min(512, w - cc * 512)
                                    o = cc * 512
                                    nc.tensor.matmul(
                                        sp[:, o:o + ww],
                                        lhsT=qg[h][:, it * 128:(it + 1) * 128],
                                        rhs=kT_t[kv][:, c * 1024 + o: c * 1024 + o + ww],
                                        start=True, stop=True)
                                if causal:
                                    dlo = nsk - 128
                                    if c * 1024 <= dlo < c * 1024 + w:
                                        o = dlo - c * 1024
                                        nc.vector.tensor_add(
                                            sp[:, o:o + 128], sp[:, o:o + 128],
                                            md[:, i * 128:(i + 1) * 128])
                                else:
                                    nc.vector.tensor_add(
                                        sp[:, :w], sp[:, :w],
                                        mrow[it][:, c * 1024: c * 1024 + w])
                                sm = p2.tile([128, 1], FP32, tag="sm", bufs=8)
                                nc.scalar.activation(
                                    prow[:, c * 1024: c * 1024 + w], sp[:, :w],
                                    mybir.ActivationFunctionType.Exp, accum_out=sm[:])
                                sums.append(sm)
                            if nch == 2:
                                tot = p2.tile([128, 1], FP32, tag="tot", bufs=4)
                                nc.vector.tensor_add(tot[:], sums[0][:], sums[1][:])
                            else:
                                tot = sums[0]
                            rc = p2.tile([128, 1], FP32, tag="rc", bufs=4)
                            nc.vector.reciprocal(rc[:], tot[:])
                            nc.vector.tensor_scalar_mul(prow[:, :nsk], prow[:, :nsk], rc[:])
                            # transpose p blocks (j <= i if causal) into quad strips
                            jtop = i if causal else ST - 1
                            for q in range(jtop // 4 + 1):
                                jlo, jhi = 4 * q, min(4 * q + 3, jtop)
                                nq = jhi - jlo + 1
                                tpp = p2ps.tile([128, 512], BF, tag="tp", bufs=2)
                                for j in range(jlo, jhi + 1):
                                    nc.tensor.transpose(
                                        tpp[:, (j - jlo) * 128:(j - jlo + 1) * 128],
                                        prow[:, j * 128:(j + 1) * 128], ident[:])
                                pt_dst = pTq[q][:].rearrange("p (a b) -> p a b", a=4)[
                                    :, 0:nq, it * 128:(it + 1) * 128]
                                pt_src = tpp[:, :nq * 128].rearrange(
                                    "p (a b) -> p a b", b=128)
                                if (it + q) % 2:
                                    nc.scalar.copy(pt_dst, pt_src)
                                else:
                                    nc.vector.tensor_copy(pt_dst, pt_src)
                        # y^T accumulation over sk-tiles
                        yp = p2ps.tile([128, 512], FP32, tag="yp", bufs=2)
                        jmax = 4 * g + 4 if causal else ST
                        for j in range(jmax):
                            lo = max(0, j - 4 * g) * 128 if causal else 0
                            nc.tensor.matmul(
                                yp[:, lo:512],
                                lhsT=v_t[j][:, kv * H:(kv + 1) * H],
                                rhs=pTq[j // 4][:, (j % 4) * 512 + lo: (j % 4) * 512 + 512],
                                start=(j == 0), stop=(j == jmax - 1))
                        nc.scalar.copy(yT_sb[h][:], yp[:])

                    # out-projection for this supertile
                    for it in range(4):
                        i = 4 * g + it
                        for dc in range(8):
                            op = p2ps.tile([128, 1024], FP32, tag="sp", bufs=2)
                            for hh in range(HEADS):
                                nc.tensor.matmul(
                                    op[:, 0:512],
                                    lhsT=yT_sb[hh][:, it * 128:(it + 1) * 128],
                                    rhs=wo_sb[hh][:, dc * 512:(dc + 1) * 512],
                                    start=(hh == 0), stop=(hh == HEADS - 1))
                            oev = p2.tile([128, 512], FP32, tag="oev", bufs=4)
                            if dc % 2:
                                nc.scalar.copy(oev[:], op[:, 0:512])
                            else:
                                nc.vector.tensor_copy(oev[:], op[:, 0:512])
                            nc.sync.dma_start(
                                cc_in[g][it * 128:(it + 1) * 128, dc * 512:(dc + 1) * 512],
                                oev[:])

                    nc.gpsimd.collective_compute(
                        "ReduceScatter", mybir.AluOpType.add,
                        replica_groups=[[0, 1, 2, 3], [4, 5, 6, 7]],
                        ins=[cc_in[g].opt()], outs=[cc_out[g].opt()])
                    nc.sync.dma_start(out_sh.ap()[g * 128:(g + 1) * 128, :],
                                      cc_out[g][:])

    nc.compile()
    return nc


_CANON_MASK = None


def _is_causal(mask: np.ndarray) -> bool:
    global _CANON_MASK
    if _CANON_MASK is None:
        _CANON_MASK = np.triu(np.full((S, S), -1e9, dtype=np.float32), k=1)
    return mask.shape == (S, S) and np.array_equal(mask, _CANON_MASK)


def _prepare(x, wq, wk, wv, wo, mask, sin, cos):
    causal = _is_causal(np.asarray(mask, dtype=np.float32))
    if causal not in _CACHE:
        _CACHE[causal] = _build(causal)
    nc = _CACHE[causal]

    x = np.asarray(x, dtype=np.float32)
    scale = np.float32(H ** -0.5)
    cosT = np.ascontiguousarray(np.asarray(cos, np.float32).T)          # [H, S]
    sinT = np.asarray(sin, np.float32).T.copy()                          # [H, S]
    sinT[0:H // 2] = -sinT[0:H // 2]                                     # signed
    # per-core weight shards; head order = r-major over local kv heads
    in_maps = []
    for c in range(N_CORES):
        b, tp = c // TP, c % TP
        ks = slice(tp * KLOC, (tp + 1) * KLOC)
        wq_c = np.asarray(wq, np.float32)[:, :, ks, :].reshape(D, HEADS * H)
        wk_c = (np.asarray(wk, np.float32)[:, ks, :] * scale).reshape(D, KLOC * H)
        wv_c = np.asarray(wv, np.float32)[:, ks, :].reshape(D, KLOC * H)
        m = {
            "xP": x[b].reshape(2, S // 2, DT, 128).transpose(3, 0, 2, 1)
                     .reshape(128, 2 * DT * (S // 2)).astype(BF16),
            "wq": wq_c.reshape(DT, 128, HEADS, H).transpose(2, 1, 0, 3)
                      .reshape(HEADS * 128, DT * 128).astype(BF16),
            "wk": wk_c.reshape(DT, 128, KLOC, H).transpose(2, 1, 0, 3)
                      .reshape(KLOC * 128, DT * 128).astype(BF16),
            "wv": wv_c.reshape(DT, 128, KLOC * H).transpose(1, 0, 2)
                      .reshape(128, DT * KLOC * H).astype(BF16),
            "wo": np.asarray(wo, np.float32)[:, ks, :, :].reshape(HEADS * H, D).astype(BF16),
            "cosT": cosT,
            "sinST": sinT,
        }
        if causal:
            md = np.empty((128, S), np.float32)
            for i in range(ST):
                md[:, i * 128:(i + 1) * 128] = mask[i * 128:(i + 1) * 128,
                                                    i * 128:(i + 1) * 128]
            m["maskd"] = md
        else:
            m["maskf"] = np.asarray(mask, np.float32)
        in_maps.append(m)
    return nc, in_maps


def _assemble(results):
    out = np.empty((B, S, D), dtype=np.float32)
    for c in range(N_CORES):
        b, tp = c // TP, c % TP
        sh = results[c]["out_shard"]
        for g in range(NG):
            out[b, g * 512 + tp * 128: g * 512 + (tp + 1) * 128, :] = \
                sh[g * 128:(g + 1) * 128]
    return out


def kernel(x, wq, wk, wv, wo, mask, sin, cos):
    nc, in_maps = _prepare(x, wq, wk, wv, wo, mask, sin, cos)
    res = bass_utils.run_bass_kernel_spmd(nc, in_maps, core_ids=list(range(N_CORES)))
    return _assemble(res.results)


def _traced_run(x, wq, wk, wv, wo, mask, sin, cos):
    """Like kernel() but with NTFF tracing; returns BassKernelResults."""
    nc, in_maps = _prepare(x, wq, wk, wv, wo, mask, sin, cos)
    res = bass_utils.run_bass_kernel_spmd(nc, in_maps, core_ids=list(range(N_CORES)),
                                          trace=True)
    res.full_output = _assemble(res.results)
    return res


# revision 9
# speedup vs baseline: 1.3713x; 1.0821x over previous
"""Tensor-parallel fused attention kernel for Trainium2 (8 NeuronCores).

Sharding: DP=2 over batch x TP=4 over kv-head pairs. Each core computes
q/k/v projections + RoPE + causal attention + output projection for its
(batch, 2 kv heads) shard in bf16, then a 4-core ReduceScatter combines
the partial output projections; the host assembles the disjoint row
shards into the full [2, 2048, 4096] output.
"""
import sys

for _p in ("/opt/trn_rl_repo", "/root/.axon_site/_ro/trn_rl_repo"):
    if _p not in sys.path:
        sys.path.append(_p)

import math
import numpy as np
import ml_dtypes

import concourse.bass as bass
import concourse.mybir as mybir
import concourse.tile as tile
from concourse import bacc
from concourse import bass_utils
from concourse.masks import make_identity

BF16 = ml_dtypes.bfloat16
FP32 = mybir.dt.float32
BF = mybir.dt.bfloat16

B, S, D = 2, 2048, 4096
R, K, H = 4, 8, 128
N_CORES = 8
TP = 4            # tensor-parallel ways (kv-head axis)
KLOC = K // TP    # kv heads per core = 2
HEADS = R * KLOC  # query heads per core = 8
DT = D // 128     # 32 d-tiles
ST = S // 128     # 16 s-tiles
NG = ST // 4      # 4 supertiles of 512 rows

_CACHE = {}


def _build(causal: bool):
    nc = bacc.Bacc("TRN2", target_bir_lowering=False, debug=False,
                   enable_asserts=False, num_devices=N_CORES)

    xP = nc.dram_tensor("xP", [128, 2 * DT * (S // 2)], BF, kind="ExternalInput")
    wq = nc.dram_tensor("wq", [HEADS * 128, DT * 128], BF, kind="ExternalInput")
    wk = nc.dram_tensor("wk", [KLOC * 128, DT * 128], BF, kind="ExternalInput")
    wv = nc.dram_tensor("wv", [128, DT * KLOC * H], BF, kind="ExternalInput")
    wo = nc.dram_tensor("wo", [HEADS * H, D], BF, kind="ExternalInput")
    cosT = nc.dram_tensor("cosT", [H, S], FP32, kind="ExternalInput")
    sinST = nc.dram_tensor("sinST", [H, S], FP32, kind="ExternalInput")
    if causal:
        maskd = nc.dram_tensor("maskd", [128, S], FP32, kind="ExternalInput")
    else:
        maskf = nc.dram_tensor("maskf", [S, S], FP32, kind="ExternalInput")
    out_sh = nc.dram_tensor("out_shard", [S // TP, D], BF, kind="ExternalOutput")

    with tile.TileContext(nc) as tc:
        with tc.tile_pool(name="persist", bufs=1) as persist, \
             tc.tile_pool(name="dram", bufs=1, space="DRAM") as dram:

            kT_t = [persist.tile([128, S], BF, tag=f"kT{i}", name=f"kT{i}")
                    for i in range(KLOC)]
            v_t = [persist.tile([128, KLOC * H], BF, tag=f"v{i}", name=f"v{i}")
                   for i in range(ST)]
            qT_dram = dram.tile([HEADS * 128, S], BF, tag="qtd", name="qT_dram")
            cc_in = [dram.tile([512, D], BF, tag=f"ccin{g}", name=f"cc_in{g}")
                     for g in range(NG)]
            cc_out = [dram.tile([128, D], BF, tag=f"ccout{g}", name=f"cc_out{g}")
                      for g in range(NG)]

            # ---------------- Phase 1: projections + rope ----------------
            with tc.tile_pool(name="p1", bufs=1) as p1, \
                 tc.tile_pool(name="p1ps", bufs=1, space="PSUM") as p1ps:
                ct = p1.tile([H, S], FP32, tag="ct")
                nc.sync.dma_start(ct[:], cosT.ap())
                st = p1.tile([H, S], FP32, tag="st")
                nc.sync.dma_start(st[:], sinST.ap())
                wv_sb = p1.tile([128, DT * KLOC * H], BF, tag="wvsb")
                nc.sync.dma_start(wv_sb[:], wv.ap())

                for half in range(2):
                    scols = (half * (S // 2), (half + 1) * (S // 2))
                    xth_t = [p1.tile([128, 8 * (S // 2)], BF, tag="xth", bufs=8,
                                     name=f"xth{half}_{qq}") for qq in range(4)]
                    for qq in range(4):
                        nc.sync.dma_start(
                            xth_t[qq][:],
                            xP.ap()[:, (half * DT + qq * 8) * (S // 2):
                                       (half * DT + (qq + 1) * 8) * (S // 2)])

                    def xth(d, a, b):
                        return xth_t[d // 8][:, (d % 8) * (S // 2) + a:
                                             (d % 8) * (S // 2) + b]

                    # q (8 head-tiles) then k (KLOC head-tiles)
                    for h in range(HEADS + KLOC):
                        wsrc = wq.ap()[h * 128:(h + 1) * 128, :] if h < HEADS \
                            else wk.ap()[(h - HEADS) * 128:(h - HEADS + 1) * 128, :]
                        wslab = p1.tile([128, DT * 128], BF, tag="wslab", bufs=2)
                        nc.sync.dma_start(wslab[:], wsrc)
                        for sc in range(2):  # 512-wide chunks within the half
                            lo = sc * 512
                            qp = p1ps.tile([128, 512], FP32, tag="qp", bufs=3)
                            for d in range(DT):
                                nc.tensor.matmul(
                                    qp[:],
                                    lhsT=wslab[:, d * 128:(d + 1) * 128],
                                    rhs=xth(d, lo, lo + 512),
                                    start=(d == 0), stop=(d == DT - 1))
                            # rope: out = qp*cos + rot(qp)*sin_signed
                            gcol = scols[0] + lo
                            t1 = p1.tile([128, 512], FP32, tag="t1", bufs=2)
                            nc.vector.tensor_mul(t1[:], qp[:], ct[:, gcol:gcol + 512])
                            t2 = p1.tile([128, 512], FP32, tag="t2", bufs=2)
                            nc.vector.tensor_mul(t2[0:64, :], qp[64:128, :],
                                                 st[0:64, gcol:gcol + 512])
                            nc.vector.tensor_mul(t2[64:128, :], qp[0:64, :],
                                                 st[64:128, gcol:gcol + 512])
                            if h < HEADS:
                                robf = p1.tile([128, 512], BF, tag="robf", bufs=2)
                                nc.vector.tensor_add(robf[:], t1[:], t2[:])
                                nc.sync.dma_start(
                                    qT_dram[h * 128:(h + 1) * 128, gcol:gcol + 512],
                                    robf[:])
                            else:
                                nc.vector.tensor_add(
                                    kT_t[h - HEADS][:, gcol:gcol + 512], t1[:], t2[:])

                    # v projection for the 8 s-tiles of this half
                    for stl in range(ST // 2):
                        sti = half * (ST // 2) + stl
                        vp = p1ps.tile([128, KLOC * H], FP32, tag="vp", bufs=2)
                        for d in range(DT):
                            nc.tensor.matmul(
                                vp[:],
                                lhsT=xth(d, stl * 128, (stl + 1) * 128),
                                rhs=wv_sb[:, d * KLOC * H:(d + 1) * KLOC * H],
                                start=(d == 0), stop=(d == DT - 1))
                        nc.scalar.copy(v_t[sti][:], vp[:])

            # ---------------- Phase 2: attention + out-proj ----------------
            with tc.tile_pool(name="p2", bufs=1) as p2, \
                 tc.tile_pool(name="p2ps", bufs=1, space="PSUM") as p2ps:
                wo_sb = [p2.tile([128, D], BF, tag=f"wo{i}", name=f"wo{i}")
                         for i in range(HEADS)]
                for i in range(HEADS):
                    nc.sync.dma_start(wo_sb[i][:], wo.ap()[i * 128:(i + 1) * 128, :])
                ident = p2.tile([128, 128], BF, tag="ident")
                make_identity(nc, ident[:])
                if causal:
                    md = p2.tile([128, S], FP32, tag="maskd")
                    nc.sync.dma_start(md[:], maskd.ap())

                for g in range(NG):
                    qg = [p2.tile([128, 512], BF, tag="qg", bufs=16, name=f"qg{g}_{h}")
                          for h in range(HEADS)]
                    for h in range(HEADS):
                        nc.sync.dma_start(
                            qg[h][:], qT_dram[h * 128:(h + 1) * 128, g * 512:(g + 1) * 512])
                    if not causal:
                        mrow = [p2.tile([128, S], FP32, tag="mrow", bufs=8,
                                        name=f"mrow{g}_{it}") for it in range(4)]
                        for it in range(4):
                            i = 4 * g + it
                            nc.sync.dma_start(mrow[it][:], maskf.ap()[i * 128:(i + 1) * 128, :])

                    yT_sb = [p2.tile([128, 512], BF, tag=f"yt{h}", bufs=2,
                                     name=f"yt{g}_{h}") for h in range(HEADS)]
                    for h in range(HEADS):
                        kv = h % KLOC
                        nquad = g + 1 if causal else NG
                        pTq = [p2.tile([128, 2048], BF, tag=f"ptq{q}", bufs=2,
                                       name=f"ptq{g}_{h}_{q}") for q in range(nquad)]
                        for it in range(4):
                            i = 4 * g + it
                            nsk = (i + 1) * 128 if causal else S
                            prow = p2.tile([128, S], BF, tag="prow", bufs=2)
                            sums = []
                            nch = (nsk + 1023) // 1024
                            for c in range(nch):
                                w = min(1024, nsk - c * 1024)
                                sp = p2ps.tile([128, 1024], FP32, tag="sp", bufs=2)
                                for cc in range((w + 511) // 512):
                                    ww = min(512, w - cc * 512)
                                    o = cc * 512
                                    nc.tensor.matmul(
                                        sp[:, o:o + ww],
                                        lhsT=qg[h][:, it * 128:(it + 1) * 128],
                                        rhs=kT_t[kv][:, c * 1024 + o: c * 1024 + o + ww],
                                        start=True, stop=True)
                                if causal:
                                    dlo = nsk - 128
                                    if c * 1024 <= dlo < c * 1024 + w:
                                        o = dlo - c * 1024
                                        nc.vector.tensor_add(
                                            sp[:, o:o + 128], sp[:, o:o + 128],
                                            md[:, i * 128:(i + 1) * 128])
                                else:
                                    nc.vector.tensor_add(
                                        sp[:, :w], sp[:, :w],
                                        mrow[it][:, c * 1024: c * 1024 + w])
                                sm = p2.tile([128, 1], FP32, tag="sm", bufs=8)
                                nc.scalar.activation(
                                    prow[:, c * 1024: c * 1024 + w], sp[:, :w],
                                    mybir.ActivationFunctionType.Exp, accum_out=sm[:])
                                sums.append(sm)
                            if nch == 2:
                                tot = p2.tile([128, 1], FP32, tag="tot", bufs=4)
                                nc.vector.tensor_add(tot[:], sums[0][:], sums[1][:])
                            else:
                                tot = sums[0]
                            rc = p2.tile([128, 1], FP32, tag="rc", bufs=4)
                            nc.vector.reciprocal(rc[:], tot[:])
                            nc.vector.tensor_scalar_mul(prow[:, :nsk], prow[:, :nsk], rc[:])
                            # transpose p blocks (j <= i if causal) into quad strips
                            jtop = i if causal else ST - 1
                            for q in range(jtop // 4 + 1):
                                jlo, jhi = 4 * q, min(4 * q + 3, jtop)
                                nq = jhi - jlo + 1
                                tpp = p2ps.tile([128, 512], BF, tag="tp", bufs=2)
                                for j in range(jlo, jhi + 1):
                                    nc.tensor.transpose(
                                        tpp[:, (j - jlo) * 128:(j - jlo + 1) * 128],
                                        prow[:, j * 128:(j + 1) * 128], ident[:])
                                pt_dst = pTq[q][:].rearrange("p (a b) -> p a b", a=4)[
                                    :, 0:nq, it * 128:(it + 1) * 128]
                                pt_src = tpp[:, :nq * 128].rearrange(
                                    "p (a b) -> p a b", b=128)
                                if (it + q) % 2:
                                    nc.scalar.copy(pt_dst, pt_src)
                                else:
                                    nc.vector.tensor_copy(pt_dst, pt_src)
                        # y^T accumulation over sk-tiles
                        yp = p2ps.tile([128, 512], FP32, tag="yp", bufs=2)
                        jmax = 4 * g + 4 if causal else ST
                        for j in range(jmax):
                            lo = max(0, j - 4 * g) * 128 if causal else 0
                            nc.tensor.matmul(
                                yp[:, lo:512],
                                lhsT=v_t[j][:, kv * H:(kv + 1) * H],
                                rhs=pTq[j // 4][:, (j % 4) * 512 + lo: (j % 4) * 512 + 512],
                                start=(j == 0), stop=(j == jmax - 1))
                        nc.scalar.copy(yT_sb[h][:], yp[:])

                    # out-projection for this supertile
                    for it in range(4):
                        i = 4 * g + it
                        for dc in range(8):
                            op = p2ps.tile([128, 1024], FP32, tag="sp", bufs=2)
                            for hh in range(HEADS):
                                nc.tensor.matmul(
                                    op[:, 0:512],
                                    lhsT=yT_sb[hh][:, it * 128:(it + 1) * 128],
                                    rhs=wo_sb[hh][:, dc * 512:(dc + 1) * 512],
                                    start=(hh == 0), stop=(hh == HEADS - 1))
                            oev = p2.tile([128, 512], BF, tag="oev", bufs=4)
                            if dc % 2:
                                nc.scalar.copy(oev[:], op[:, 0:512])
                            else:
                                nc.vector.tensor_copy(oev[:], op[:, 0:512])
                            nc.sync.dma_start(
                                cc_in[g][it * 128:(it + 1) * 128, dc * 512:(dc + 1) * 512],
                                oev[:])

                    nc.gpsimd.collective_compute(
                        "ReduceScatter", mybir.AluOpType.add,
                        replica_groups=[[0, 1, 2, 3], [4, 5, 6, 7]],
                        ins=[cc_in[g].opt()], outs=[cc_out[g].opt()])
                    nc.sync.dma_start(out_sh.ap()[g * 128:(g + 1) * 128, :],
                                      cc_out[g][:])

    nc.compile()
    return nc


_CANON_MASK = None


def _is_causal(mask: np.ndarray) -> bool:
    global _CANON_MASK
    if _CANON_MASK is None:
        _CANON_MASK = np.triu(np.full((S, S), -1e9, dtype=np.float32), k=1)
    return mask.shape == (S, S) and np.array_equal(mask, _CANON_MASK)


def _prepare(x, wq, wk, wv, wo, mask, sin, cos):
    causal = _is_causal(np.asarray(mask, dtype=np.float32))
    if causal not in _CACHE:
        _CACHE[causal] = _build(causal)
    nc = _CACHE[causal]

    x = np.asarray(x, dtype=np.float32)
    scale = np.float32(H ** -0.5)
    cosT = np.ascontiguousarray(np.asarray(cos, np.float32).T)          # [H, S]
    sinT = np.asarray(sin, np.float32).T.copy()                          # [H, S]
    sinT[0:H // 2] = -sinT[0:H // 2]                                     # signed
    # per-core weight shards; head order = r-major over local kv heads
    in_maps = []
    for c in range(N_CORES):
        b, tp = c // TP, c % TP
        ks = slice(tp * KLOC, (tp + 1) * KLOC)
        wq_c = np.asarray(wq, np.float32)[:, :, ks, :].reshape(D, HEADS * H)
        wk_c = (np.asarray(wk, np.float32)[:, ks, :] * scale).reshape(D, KLOC * H)
        wv_c = np.asarray(wv, np.float32)[:, ks, :].reshape(D, KLOC * H)
        m = {
            "xP": x[b].reshape(2, S // 2, DT, 128).transpose(3, 0, 2, 1)
                     .reshape(128, 2 * DT * (S // 2)).astype(BF16),
            "wq": wq_c.reshape(DT, 128, HEADS, H).transpose(2, 1, 0, 3)
                      .reshape(HEADS * 128, DT * 128).astype(BF16),
            "wk": wk_c.reshape(DT, 128, KLOC, H).transpose(2, 1, 0, 3)
                      .reshape(KLOC * 128, DT * 128).astype(BF16),
            "wv": wv_c.reshape(DT, 128, KLOC * H).transpose(1, 0, 2)
                      .reshape(128, DT * KLOC * H).astype(BF16),
            "wo": np.asarray(wo, np.float32)[:, ks, :, :].reshape(HEADS * H, D).astype(BF16),
            "cosT": cosT,
            "sinST": sinT,
        }
        if causal:
            md = np.empty((128, S), np.float32)
            for i in range(ST):
                md[:, i * 128:(i + 1) * 128] = mask[i * 128:(i + 1) * 128,
                                                    i * 128:(i + 1) * 128]
            m["maskd"] = md
        else:
            m["maskf"] = np.asarray(mask, np.float32)
        in_maps.append(m)
    return nc, in_maps


def _assemble(results):
    out = np.empty((B, S, D), dtype=np.float32)
    for c in range(N_CORES):
        b, tp = c // TP, c % TP
        sh = results[c]["out_shard"].astype(np.float32)
        for g in range(NG):
            out[b, g * 512 + tp * 128: g * 512 + (tp + 1) * 128, :] = \
                sh[g * 128:(g + 1) * 128]
    return out


def kernel(x, wq, wk, wv, wo, mask, sin, cos):
    nc, in_maps = _prepare(x, wq, wk, wv, wo, mask, sin, cos)
    res = bass_utils.run_bass_kernel_spmd(nc, in_maps, core_ids=list(range(N_CORES)))
    return _assemble(res.results)


def _traced_run(x, wq, wk, wv, wo, mask, sin, cos):
    """Like kernel() but with NTFF tracing; returns BassKernelResults."""
    nc, in_maps = _prepare(x, wq, wk, wv, wo, mask, sin, cos)
    res = bass_utils.run_bass_kernel_spmd(nc, in_maps, core_ids=list(range(N_CORES)),
                                          trace=True)
    res.full_output = _assemble(res.results)
    return res
